# revision 3
# baseline (speedup 1.0000x reference)
"""Trainium2 Bass kernel for nn_NodeFeat_77841987272986 (2-hop GNN message passing).

v5: bf16 one-hot matmul SpMM with chunk-major tables and overlapped AllGather.

  - 1-D node-parallel over 8 cores (12500 dest rows each); edges sharded by
    dest, sorted by (dest-tile, col-window, col).
  - SpMM via one-hot matmuls: per 128-edge chunk, lhsT is a [128e x 128d]
    bf16 one-hot built on DVE (layout [e, d, chunk] keeps every operand
    2-byte & packed -> 2x DVE mode), rhs is the dma-gathered source rows.
  - Two PSUM accumulation chains per dest tile (psA 128 cols / psB 64 cols)
    in separate banks -- interleaved chains in one bank corrupt results.
  - hop1 gathers 256B rows from t1 = [x*rsqrt(deg), x*sqrt(deg)] (fully
    useful); the x-block is recovered with an s-weighted one-hot since
    (x*rsqrt(deg))*sqrt(deg) = x.
  - hop2 gathers 512B rows from the padded bf16 y1 table.
  - The node space is relabeled chunk-major: 4 AllGather chunks, each a
    contiguous Shared tensor of <=32768 rows that doubles as one int16
    gather window. AG chunk k is issued as soon as the hop-1 tiles covering
    its rows are stored, overlapping the collective with the rest of hop 1.
"""
import sys
sys.path.insert(0, '/opt/trn_rl_repo')
import numpy as np

P = 128            # partitions / dest-tile size / edge-chunk size
GROUP = 4          # dest tiles per gather group
NCH = 4            # AllGather chunks == gather windows

FULL_N = 100000
FULL_D = 64
FULL_CORES = 8


# ----------------------------------------------------------------- host prep

def _plan(N, D, n_cores, row, col, deg, x, group=GROUP):
    npc = N // n_cores
    n_tiles = (npc + P - 1) // P
    n_groups = (n_tiles + group - 1) // group

    # chunk-major node relabeling: tile ranges per AG chunk
    tpc = (n_tiles + NCH - 1) // NCH                  # tiles per chunk
    tile_lo = [min(n_tiles, tpc * i) for i in range(NCH + 1)]
    row_lo = [min(npc, P * t) for t in tile_lo]       # per-core row bounds
    rck = [row_lo[i + 1] - row_lo[i] for i in range(NCH)]   # rows per chunk
    win_rows = [n_cores * r for r in rck]             # window sizes (<=32768)
    assert all(w <= 32768 for w in win_rows)
    woff = np.cumsum([0] + win_rows)                  # chunk-major offsets

    deg = deg.reshape(-1).astype(np.float64)
    sr = (1.0 / np.sqrt(deg)).astype(np.float32)
    sq = np.sqrt(deg).astype(np.float32)
    dr = (1.0 / deg).astype(np.float32)
    import ml_dtypes
    bf16 = ml_dtypes.bfloat16
    x3f = np.concatenate([x, x * sr[:, None], x * sq[:, None]], axis=1)
    x3f = np.ascontiguousarray(x3f, dtype=np.float32)

    # node -> (window b, position inside window)
    nodes = np.arange(N)
    ncore = nodes // npc
    nrow = nodes % npc
    nch = np.minimum(np.searchsorted(row_lo, nrow, side='right') - 1, NCH - 1)
    npos = np.asarray(rck)[nch] * ncore + (nrow - np.asarray(row_lo)[nch])
    gpos = woff[nch] + npos                           # global chunk-major pos

    # hop-1 gather table in chunk-major order [N, 128] bf16 (256B rows)
    t1 = np.empty((N, 2 * D), dtype=bf16)
    t1[gpos] = x3f[:, D:3 * D].astype(bf16)

    core_of = row // npc
    per_core = []
    cnts = np.zeros((n_cores, n_tiles, NCH), dtype=np.int64)
    for c in range(n_cores):
        m = core_of == c
        er = row[m] - c * npc
        ec = col[m]
        t = er // P
        b = nch[ec]
        order = np.lexsort((ec, b, t))
        er, ec, t, b = er[order], ec[order], t[order], b[order]
        np.add.at(cnts[c], (t, b), 1)
        per_core.append((er, ec, t, b))

    K = np.ceil(cnts.max(axis=0) / P).astype(np.int64)          # [n_tiles, NCH]
    for t in range(n_tiles):
        if K[t].sum() == 0:
            K[t][0] = 1

    groups = []      # (g, tiles_g, [(b, [(t, k, pos0)...], gb_pos0, gb_K)])
    pos = 0
    for g in range(n_groups):
        tiles_g = list(range(g * group, min((g + 1) * group, n_tiles)))
        bl = []
        for b in range(NCH):
            ent = []
            gb_pos0 = pos
            for t in tiles_g:
                k = int(K[t][b])
                if k:
                    ent.append((t, k, pos))
                    pos += k
            bl.append((b, ent, gb_pos0, pos - gb_pos0))
        groups.append((g, tiles_g, bl))
    CHT = pos

    ins = []
    for c in range(n_cores):
        er, ec, t, b = per_core[c]
        idx = np.zeros(CHT * P, dtype=np.int16)
        rowmod = np.full(CHT * P, -1.0, dtype=np.float32)
        scol = np.zeros(CHT * P, dtype=np.float32)
        starts = np.searchsorted(t * NCH + b, np.arange(n_tiles * NCH))
        counts = np.diff(np.append(starts, len(t)))
        for g, tiles_g, bl in groups:
            for bb, ent, gb_pos0, gb_K in bl:
                for (tt, k, pos0) in ent:
                    s = starts[tt * NCH + bb]
                    n = counts[tt * NCH + bb]
                    assert n <= k * P
                    sl = slice(pos0 * P, pos0 * P + n)
                    ecs = ec[s:s + n]
                    idx[sl] = (gpos[ecs] - woff[bb]).astype(np.int16)
                    rowmod[sl] = (er[s:s + n] - tt * P).astype(np.float32)
                    scol[sl] = sq[ecs]
        iw = idx.reshape(CHT, 8, 16).transpose(2, 0, 1)          # [16, CHT, 8]
        iw = np.tile(iw.reshape(16, CHT * 8), (8, 1))            # [128, CHT*8]
        rm = rowmod.reshape(CHT, P).T.astype(bf16)               # [128, CHT]
        sc = scol.reshape(CHT, P).T.astype(bf16)                 # [128, CHT]
        base = c * npc
        tmp = np.ones(n_tiles * P, dtype=np.float32)
        tmp[:npc] = dr[base:base + npc]
        drw = tmp.reshape(n_tiles, P).T.copy()
        x3o = np.ascontiguousarray(x3f[base:base + npc])
        ins.append({
            "idx16": np.ascontiguousarray(iw),
            "rowmod": np.ascontiguousarray(rm),
            "scol": np.ascontiguousarray(sc),
            "drw": drw,
            "x3o": x3o,
            "t1": t1,
        })
    static = dict(N=N, D=D, n_cores=n_cores, npc=npc, n_tiles=n_tiles,
                  groups=groups, K=K, CHT=CHT, group=group,
                  tile_lo=tile_lo, row_lo=row_lo, rck=rck,
                  win_rows=win_rows, woff=[int(v) for v in woff])
    return static, ins


# ------------------------------------------------------------- device kernel

def _build(static, reps=1, variant='full', single_packet=False):
    import concourse.bass as bass
    import concourse.bacc as bacc
    import concourse.mybir as mybir
    import concourse.tile as tile

    N = static["N"]; D = static["D"]; n_cores = static["n_cores"]
    npc = static["npc"]; n_tiles = static["n_tiles"]
    groups = static["groups"]; CHT = static["CHT"]
    tile_lo = static["tile_lo"]; row_lo = static["row_lo"]
    win_rows = static["win_rows"]; woff = static["woff"]
    D3 = 3 * D
    f32 = mybir.dt.float32
    bf16 = mybir.dt.bfloat16
    W1 = 2 * D       # hop-1 table width (bf16) -> 256B rows
    W2 = 256         # hop-2 padded table width (bf16) -> 512B rows

    KGB = max(gbk for _, _, bl in groups for (_, ent, _, gbk) in bl)

    nc = bacc.Bacc("TRN2", target_bir_lowering=False, debug=False,
                   num_devices=n_cores, num_swdge_queues=4)
    t1_d = nc.dram_tensor("t1", [N, W1], bf16, kind="ExternalInput")
    x3o_d = nc.dram_tensor("x3o", [npc, D3], f32, kind="ExternalInput")
    idx16_d = nc.dram_tensor("idx16", [P, CHT * 8], mybir.dt.int16,
                             kind="ExternalInput")
    rowmod_d = nc.dram_tensor("rowmod", [P, CHT], bf16, kind="ExternalInput")
    scol_d = nc.dram_tensor("scol", [P, CHT], bf16, kind="ExternalInput")
    drw_d = nc.dram_tensor("drw", [P, n_tiles], f32, kind="ExternalInput")
    out_d = nc.dram_tensor("out", [npc, 9 * D], f32, kind="ExternalOutput")
    y1sh = nc.dram_tensor("y1sh", [npc, W2], bf16)
    shared = "Shared" if n_cores > 1 else "Local"
    y1f = [nc.dram_tensor(f"y1f{k}", [win_rows[k], W2], bf16,
                          addr_space=shared) for k in range(NCH)]

    def rows_of(t):
        return min(P, npc - t * P)

    with tile.TileContext(nc) as tc:
        with tc.tile_pool(name="res", bufs=1) as res, \
             tc.tile_pool(name="gat", bufs=4) as gat, \
             tc.tile_pool(name="oh", bufs=3) as ohp, \
             tc.tile_pool(name="ep", bufs=3) as ep, \
             tc.tile_pool(name="ps", bufs=4, space="PSUM") as psp:

            idx16 = res.tile([P, CHT * 8], mybir.dt.int16)
            nc.sync.dma_start(out=idx16[:], in_=idx16_d[:])
            rowmod = res.tile([P, CHT], bf16)
            nc.sync.dma_start(out=rowmod[:], in_=rowmod_d[:])
            scol = res.tile([P, CHT], bf16)
            nc.sync.dma_start(out=scol[:], in_=scol_d[:])
            drw = res.tile([P, n_tiles], f32)
            nc.sync.dma_start(out=drw[:], in_=drw_d[:])
            # iota_rep[p, d*KGB + k] = d  (bf16), for the [e, d, k] one-hot
            iota_i = res.tile([P, P], mybir.dt.int32)
            nc.gpsimd.iota(iota_i[:], pattern=[[1, P]], base=0,
                           channel_multiplier=0)
            iota_rep = res.tile([P, P * KGB], bf16)
            src = bass.AP(iota_i.tensor, iota_i[:].offset,
                          [iota_i[:].ap[0], [1, P], [0, KGB]])
            nc.vector.tensor_copy(out=iota_rep[:].rearrange(
                "p (d k) -> p d k", d=P), in_=src)

            def hop(tables, Wt, weighted, store, rhs_tag, split_cb=None):
                # tables: list of per-window APs (chunk-major node order)
                for g, tiles_g, bl in groups:
                    psums = {}
                    if variant != 'gathers':
                        for t in tiles_g:
                            psums[t] = (psp.tile([P, 2 * D], f32, tag="psA",
                                                 name=f"psA{t}"),
                                        psp.tile([P, D], f32, tag="psB",
                                                 name=f"psB{t}"))
                    first = {t: True for t in tiles_g}
                    firstx = {t: True for t in tiles_g}
                    last_pos = {}
                    for b, ent, gb_pos0, gb_K in bl:
                        for (t, k, pos0) in ent:
                            last_pos[t] = pos0 + k - 1
                    for b, ent, gb_pos0, gb_K in bl:
                        if gb_K == 0:
                            continue
                        ni = gb_K * P
                        rhs = gat.tile([P, KGB, Wt], bf16, tag=rhs_tag)
                        nc.gpsimd.dma_gather(
                            out_ap=rhs[:, 0:gb_K, :],
                            in_ap=tables[b],
                            idxs_ap=idx16[:, (gb_pos0 * 8):
                                          (gb_pos0 * 8 + ni // 16)],
                            num_idxs=ni, num_idxs_reg=ni,
                            elem_size=Wt, single_packet=single_packet,
                            queue_num=b % 4)
                        if variant == 'gathers':
                            continue
                        oht = ohp.tile([P, P, KGB], bf16, tag="oh")
                        rm_sl = rowmod[:, gb_pos0:gb_pos0 + gb_K]
                        in0 = bass.AP(iota_rep.tensor, iota_rep[:].offset,
                                      [iota_rep[:].ap[0], [KGB, P], [1, gb_K]])
                        in1 = bass.AP(rowmod.tensor, rm_sl.offset,
                                      [rm_sl.ap[0], [0, P],
                                       [rm_sl.ap[-1][0], gb_K]])
                        nc.vector.tensor_tensor(
                            out=oht[:, :, 0:gb_K], in0=in0, in1=in1,
                            op=mybir.AluOpType.is_equal)
                        if weighted:
                            ohs = ohp.tile([P, P, KGB], bf16, tag="ohs")
                            sc_sl = scol[:, gb_pos0:gb_pos0 + gb_K]
                            in1s = bass.AP(scol.tensor, sc_sl.offset,
                                           [sc_sl.ap[0], [0, P],
                                            [sc_sl.ap[-1][0], gb_K]])
                            nc.vector.tensor_tensor(
                                out=ohs[:, :, 0:gb_K], in0=oht[:, :, 0:gb_K],
                                in1=in1s, op=mybir.AluOpType.mult)
                        for (t, k, pos0) in ent:
                            for j in range(k):
                                pos = pos0 + j
                                kk = pos - gb_pos0
                                psA, psB = psums[t]
                                nc.tensor.matmul(
                                    out=psA[:],
                                    lhsT=oht[:, :, kk],
                                    rhs=rhs[:, kk, 0:2 * D],
                                    start=first[t],
                                    stop=(pos == last_pos[t]))
                                first[t] = False
                                nc.tensor.matmul(
                                    out=psB[:],
                                    lhsT=(ohs if weighted else oht)[:, :, kk],
                                    rhs=rhs[:, kk, 0:D] if weighted
                                        else rhs[:, kk, 2 * D:D3],
                                    start=firstx[t],
                                    stop=(pos == last_pos[t]))
                                firstx[t] = False
                    if variant == 'gathers':
                        continue
                    for t in tiles_g:
                        store(t, psums[t])
                        if split_cb is not None and (t + 1) in tile_lo:
                            split_cb(tile_lo.index(t + 1) - 1)

            # ---- hop 1 epilogue: y1 = psum * (1/deg)  -> y1sh (bf16 padded)
            def store1(t, ps):
                psA, psB = ps
                r = rows_of(t)
                y1d = ep.tile([P, W2], bf16, tag="y1d")
                nc.vector.memset(y1d[:, D3:W2], 0.0)
                # weighted hop1: psB = x-block (cols 0:D), psA = (xr, xs)
                nc.vector.tensor_scalar(
                    out=y1d[:, 0:D], in0=psB[:], scalar1=drw[:, t:t + 1],
                    scalar2=None, op0=mybir.AluOpType.mult)
                nc.vector.tensor_scalar(
                    out=y1d[:, D:D3], in0=psA[:], scalar1=drw[:, t:t + 1],
                    scalar2=None, op0=mybir.AluOpType.mult)
                nc.scalar.dma_start(out=y1sh[t * P: t * P + r, :],
                                    in_=y1d[:r, :])

            def ag_chunk(k):
                r0, r1 = row_lo[k], row_lo[k + 1]
                if n_cores > 1:
                    nc.gpsimd.collective_compute(
                        "AllGather", mybir.AluOpType.bypass,
                        replica_groups=[list(range(n_cores))],
                        ins=[y1sh[r0:r1, :]], outs=[y1f[k][:]])
                else:
                    nc.sync.dma_start(out=y1f[k][:], in_=y1sh[r0:r1, :])

            # ---- hop 2 epilogue: y2 = psum*(1/deg) - x3 ; assemble output
            def store2(t, ps):
                psA, psB = ps
                r = rows_of(t)
                x3t = ep.tile([P, D3], f32, tag="x3t")
                nc.sync.dma_start(out=x3t[:r, :],
                                  in_=x3o_d[t * P: t * P + r, :])
                y1t = ep.tile([P, W2], bf16, tag="y1t")
                nc.sync.dma_start(out=y1t[:r, :],
                                  in_=y1sh[t * P: t * P + r, :])
                y2d = ep.tile([P, D3], f32, tag="y2d")
                # unweighted hop2: psA = y2 cols 0:2D, psB = cols 2D:3D
                nc.vector.tensor_scalar(
                    out=y2d[:, 0:2 * D], in0=psA[:], scalar1=drw[:, t:t + 1],
                    scalar2=None, op0=mybir.AluOpType.mult)
                nc.vector.tensor_scalar(
                    out=y2d[:, 2 * D:D3], in0=psB[:], scalar1=drw[:, t:t + 1],
                    scalar2=None, op0=mybir.AluOpType.mult)
                nc.vector.tensor_tensor(out=y2d[:r, :], in0=y2d[:r, :],
                                        in1=x3t[:r, :],
                                        op=mybir.AluOpType.subtract)
                ot = ep.tile([P, D, 9], f32, tag="ot")
                nc.vector.tensor_copy(
                    out=ot[:r, :, 0:3],
                    in_=x3t[:r, :].rearrange("p (v f) -> p f v", v=3))
                nc.vector.tensor_copy(
                    out=ot[:r, :, 3:6],
                    in_=y1t[:r, 0:D3].rearrange("p (v f) -> p f v", v=3))
                nc.vector.tensor_copy(
                    out=ot[:r, :, 6:9],
                    in_=y2d[:r, :].rearrange("p (v f) -> p f v", v=3))
                nc.scalar.dma_start(
                    out=out_d[t * P: t * P + r, :],
                    in_=ot[:r, :, :].rearrange("p f v -> p (f v)"))

            def store_g(t, ps):
                pass

            y1_tables = [y1f[b][:] for b in range(NCH)]
            t1_tables = [t1_d[woff[b]:woff[b] + win_rows[b], :]
                         for b in range(NCH)]
            for _rep in range(reps):
                if variant == 'gathers':
                    hop(t1_tables, W1, False, store_g, "rhs1")
                    hop(y1_tables, W2, False, store_g, "rhs2")
                    if _rep == 0:
                        z = ep.tile([P, D3], f32, tag="y2d")
                        nc.vector.memset(z[:], 0.0)
                        nc.sync.dma_start(out=out_d[0:P, 0:D3], in_=z[:])
                    continue
                hop(t1_tables, W1, True, store1, "rhs1", split_cb=ag_chunk)
                if variant != 'hop1':
                    hop(y1_tables, W2, False, store2, "rhs2")
                elif _rep == 0:
                    z = ep.tile([P, D3], f32, tag="y2d")
                    nc.vector.memset(z[:], 0.0)
                    nc.sync.dma_start(out=out_d[0:P, 0:D3], in_=z[:])

    nc.compile()
    return nc


# ----------------------------------------------------------------- interface

_CACHE = {}

def _get_nc(static):
    key = (static["N"], static["D"], static["n_cores"], static["CHT"],
           tuple(static["K"].reshape(-1).tolist()))
    if key not in _CACHE:
        _CACHE[key] = _build(static)
    return _CACHE[key]


def kernel(x, deg, row, col):
    from concourse.bass_utils import run_bass_kernel_spmd
    x = np.asarray(x, dtype=np.float32)
    deg = np.asarray(deg, dtype=np.float32)
    row = np.asarray(row).astype(np.int64)
    col = np.asarray(col).astype(np.int64)
    N, D = x.shape
    n_cores = FULL_CORES
    static, ins = _plan(N, D, n_cores, row, col, deg, x)
    nc = _get_nc(static)
    in_maps = [{"t1": m["t1"], "x3o": m["x3o"], "idx16": m["idx16"],
                "rowmod": m["rowmod"], "scol": m["scol"], "drw": m["drw"]}
               for m in ins]
    try:
        res = run_bass_kernel_spmd(nc, in_maps, core_ids=list(range(n_cores)))
    except Exception:
        res = run_bass_kernel_spmd(nc, in_maps, core_ids=list(range(n_cores)))
    out = np.concatenate([res.results[c]["out"] for c in range(n_cores)],
                         axis=0)
    return out.reshape(N, D, 9).astype(np.float32)
